# revision 5
# baseline (speedup 1.0000x reference)
"""CrossHeadProjectionV2 Trainium2 kernel.

out[b,n,t,s] = sum_m x[m,t,s] * (W_t + C_s)[m,n]
  W_t = (w + I) + qw1[t]^T qw2[t] + diag(qdd[t])   (host-folded, per-t 16x16)
  C_s = kw1[s]^T kw2[s] + diag(kdd[s])             (per-s 16x16, rank-2 + diag)

Shard T over 8 cores (256 t each). Tiles of 8 t's x 16 heads = 128
partitions, S=2048 free; pipeline unit = half tile (1024 cols = 2 psum
chunks). Per half:
  PE:  po        = Wbd^T x            (block-diag 16x16 per t, 2 mms)
       ph[i]     = Rep^T (x*kw1r_i)   (group-sum + n-broadcast, 4 mms)
       po       += Id^T u_i, Id^T tmp2  (6 mms)
  DVE: tmp_i = x*kw1r_i ; u_i = h_i*kw2r_i   (bf16 2x TT)
  ACT: psum evacuations (ph -> sbuf bf16, po -> out sbuf bf16)
Software-pipelined: stage B (idents/out-copy/dma-out) of half k-1 is
emitted after stage A of half k so every engine FIFO has ready work.

DMA layer (v2): x and out live in DRAM partition-major ([128, nt, S],
host packs/unpacks) so loads run 4 tiles per dma_start with 16 KB
contiguous descriptors and stores run 2 tiles per dma_start with 8 KB
descriptors. Stores + consts issue on the scalar HWDGE ring
(qActDynamicHW), loads on the sync ring (qSPDynamicHW): both rings'
packets spread across all 16 SDMA engines instead of piling onto 8.
wbd is also packed [128, nt*128] (8 KB rows vs 4096x 256 B descs).
"""

import numpy as np
import ml_dtypes

bf = ml_dtypes.bfloat16

B, N, T, S = 1, 16, 2048, 2048
G, I, M = 1, 2, 16
NCORES = 8
TC = T // NCORES        # 256 t per core
TB = 8                  # t rows per tile (TB*M = 128 partitions)
SC = 512                # psum chunk (one fp32 bank)
HW = 2 * SC             # half-tile width
XCHUNK = 4              # x tiles loaded per dma_start (16 KB descs)
OCHUNK = 2              # out tiles stored per dma_start (8 KB descs)

# engine assignment knobs
TMP2_ENGINE = "vector"   # GPSIMD shares DVE's SBUF port: keep Pool idle
OUTCOPY_ENGINES = ("scalar", "scalar")
BUFS = dict(xp=3, tmpp=8, t2p=8, hp=8, up=8, op=3, ps_o=4, ps_h=2)
U_ROUTE = "act2"      # ph evac on ACT (own port), DVE muls at 2x
REPS_FIRST = True     # emit rep matmuls before mains (unblocks DVE/ACT chain)
UC1_AFTER_TMP = False  # scheduling edge: next tmps before u_c1 on DVE

_cache = {}


def _build(tc_size=TC, reps=1, inner=1):
    import contextlib
    import concourse.mybir as mybir
    import concourse.tile as tile
    from concourse import bacc

    bf16, f32 = mybir.dt.bfloat16, mybir.dt.float32
    nt = tc_size // TB
    nh = nt * 2

    nc = bacc.Bacc("TRN2", target_bir_lowering=False, debug=False)

    x_d = nc.dram_tensor("x", [128, nt, S], bf16, kind="ExternalInput")
    wbd_d = nc.dram_tensor("wbd", [128, nt, 128], bf16, kind="ExternalInput")
    rep_d = nc.dram_tensor("rep", [128, 128], bf16, kind="ExternalInput")
    idn_d = nc.dram_tensor("idn", [128, 128], bf16, kind="ExternalInput")
    ewin_d = nc.dram_tensor("ewin", [128, 3, S], bf16, kind="ExternalInput")
    ewout_d = nc.dram_tensor("ewout", [128, 2, S], bf16, kind="ExternalInput")
    out_d = nc.dram_tensor("out", [128, nt, S], bf16, kind="ExternalOutput")

    def eng(name):
        return getattr(nc, {"scalar": "scalar", "vector": "vector",
                            "gpsimd": "gpsimd"}[name])

    def copy_on(engine_name, dst, src):
        if engine_name == "scalar":
            nc.scalar.copy(dst, src)
        else:
            nc.vector.tensor_copy(dst, src)

    with tile.TileContext(nc) as tc:
        with (
            tc.tile_pool(name="const", bufs=1) as constp,
            tc.tile_pool(name="xp", bufs=BUFS["xp"]) as xp,
            tc.tile_pool(name="tmpp", bufs=BUFS["tmpp"]) as tmpp,
            tc.tile_pool(name="t2p", bufs=BUFS["t2p"]) as t2p,
            tc.tile_pool(name="hp", bufs=BUFS["hp"]) as hp,
            tc.tile_pool(name="up", bufs=BUFS["up"]) as up,
            tc.tile_pool(name="op", bufs=BUFS["op"]) as op,
            tc.tile_pool(name="ps_o", bufs=BUFS["ps_o"], space="PSUM") as ps_o,
            tc.tile_pool(name="ps_h", bufs=BUFS["ps_h"], space="PSUM") as ps_h,
        ):
            # Loads ordered by first use: ewin[:, 0:2] feeds the first tmp
            # ops, then the x chunks; everything else arrives behind them
            # so the pipeline ramps fast. Consts ride the scalar ring.
            ewin = constp.tile([128, 3, S], bf16)
            nc.scalar.dma_start(ewin[:, 0:2], ewin_d.ap()[:, 0:2])
            rep = constp.tile([128, 128], bf16)
            wbd = constp.tile([128, nt, 128], bf16)
            idn = constp.tile([128, 128], bf16)
            ewout = constp.tile([128, 2, S], bf16)

            def load_late_consts():
                nc.scalar.dma_start(rep[:], rep_d.ap())
                nc.scalar.dma_start(ewin[:, 2:3], ewin_d.ap()[:, 2:3])
                nc.scalar.dma_start(ewout[:], ewout_d.ap())
                nc.scalar.dma_start(idn[:], idn_d.ap())
                nc.scalar.dma_start(
                    wbd[:].rearrange("p t q -> p (t q)"),
                    wbd_d.ap().rearrange("p t q -> p (t q)"))

            chunk_tiles = {}   # chunk idx -> [128, XCHUNK*S] tile
            tmps = {}   # half -> (tmp, tmp2), produced one iteration early
            state = {}  # half -> dict for stage B
            late_consts_done = []
            pending_uc1 = []  # last u_c1 instr, to order next tmps ahead
            out_tiles = {}  # out pair idx -> [128, OCHUNK, S] tile

            nchunks = (nt + XCHUNK - 1) // XCHUNK

            def load_chunk(ci):
                if ci in chunk_tiles or ci >= nchunks:
                    return
                xt = xp.tile([128, XCHUNK, S], bf16)
                nc.sync.dma_start(
                    xt[:].rearrange("p k s -> p (k s)"),
                    x_d.ap()[:, ci * XCHUNK:(ci + 1) * XCHUNK, :].rearrange(
                        "p k s -> p (k s)"))
                chunk_tiles[ci] = xt

            def x_of_tile(ti):
                return chunk_tiles[ti // XCHUNK][:, ti % XCHUNK]

            def stage_tmp(h):
                """Elementwise pre-multiplies for half h (emitted one
                iteration ahead so DVE never waits on this iteration's PE)."""
                ti, half = divmod(h, 2)
                hs = slice(half * HW, (half + 1) * HW)
                load_chunk(ti // XCHUNK)
                load_chunk(ti // XCHUNK + 1)
                load_chunk(ti // XCHUNK + 2)
                xt = x_of_tile(ti)
                tmp = tmpp.tile([128, 2, HW], bf16)
                tmp_instrs = [
                    nc.vector.tensor_mul(tmp[:, 0], xt[:, hs], ewin[:, 0, hs]),
                    nc.vector.tensor_mul(tmp[:, 1], xt[:, hs], ewin[:, 1, hs]),
                ]
                if UC1_AFTER_TMP and pending_uc1:
                    from concourse.tile import add_dep_helper
                    uc1 = pending_uc1.pop()
                    for tin in tmp_instrs:
                        add_dep_helper(uc1.ins, tin.ins, sync=False,
                                       reason="order next tmps before u_c1")
                if not late_consts_done:
                    late_consts_done.append(True)
                    load_late_consts()
                tmp2 = t2p.tile([128, HW], bf16)
                eng(TMP2_ENGINE).tensor_mul(tmp2[:], xt[:, hs],
                                            ewin[:, 2, hs])
                tmps[h] = dict(tmp=tmp, tmp2=tmp2, off=0)

            def stage_mm(h):
                ti, half = divmod(h, 2)
                xt = x_of_tile(ti)
                tdict = tmps.pop(h)
                tmp, tmp2, toff = tdict["tmp"], tdict["tmp2"], tdict["off"]

                def emit_mains():
                    po_cs = []
                    for c in range(2):
                        cs = slice(half * HW + c * SC,
                                   half * HW + (c + 1) * SC)
                        po_c = ps_o.tile([128, SC], f32)
                        po_cs.append(po_c)
                        nc.tensor.matmul(po_c[:], wbd[:, ti], xt[:, cs],
                                         start=True, stop=False)
                    return po_cs

                def emit_reps():
                    phs = []
                    for c in range(2):
                        ph = ps_h.tile([128, 2, SC], f32)
                        for i in range(2):
                            nc.tensor.matmul(
                                ph[:, i], rep[:],
                                tmp[:, i, toff + c * SC:toff + (c + 1) * SC],
                                start=True, stop=True)
                        phs.append(ph)
                    return phs

                if REPS_FIRST:
                    phs = emit_reps()
                    po_cs = emit_mains()
                else:
                    po_cs = emit_mains()
                    phs = emit_reps()
                state[h] = dict(ti=ti, half=half, po_cs=po_cs, phs=phs,
                                tmp2=tmp2, toff=toff)

            def stage_u(h):
                st = state[h]
                half, phs = st["half"], st.pop("phs")
                u = up.tile([128, 2, HW], bf16)
                hs = slice(half * HW, (half + 1) * HW)
                # "act2": per-chunk ACT evac + DVE 2x mul (GPS idle)
                for c in range(2):
                    cslc = slice(half * HW + c * SC, half * HW + (c + 1) * SC)
                    hsb = hp.tile([128, 2, SC], bf16)
                    nc.scalar.copy(hsb[:], phs[c][:])
                    uc1 = nc.vector.tensor_mul(
                        u[:, :, c * SC:(c + 1) * SC], hsb[:],
                        ewout[:, :, cslc])
                if UC1_AFTER_TMP:
                    pending_uc1.clear()
                    pending_uc1.append(uc1)
                st["u"] = u

            def stage_b(h):
                st = state.pop(h)
                ti, half, po_cs, tmp2 = (st["ti"], st["half"], st["po_cs"],
                                         st["tmp2"])
                u = st.get("u")
                toff = st["toff"]
                pi, pk = divmod(ti, OCHUNK)
                if pi not in out_tiles:
                    out_tiles[pi] = op.tile([128, OCHUNK, S], bf16, name="ot")
                ot = out_tiles[pi]
                for c in range(2):
                    csl = slice(c * SC, (c + 1) * SC)
                    po_c = po_cs[c][:]
                    u0 = u[:, 0, csl]
                    u1 = u[:, 1, csl]
                    nc.tensor.matmul(po_c, idn[:], u0,
                                     start=False, stop=False)
                    nc.tensor.matmul(po_c, idn[:], u1,
                                     start=False, stop=False)
                    nc.tensor.matmul(po_c, idn[:],
                                     tmp2[:, toff + c * SC:
                                          toff + (c + 1) * SC],
                                     start=False, stop=True)
                    ename = OUTCOPY_ENGINES[(2 * half + c) % 2]
                    copy_on(ename, ot[:, pk, half * HW + c * SC:
                                      half * HW + (c + 1) * SC], po_c)
                if pk == OCHUNK - 1 and half == 1:
                    # pair complete: store OCHUNK tiles in one dma (8 KB
                    # descriptors), on the scalar ring behind its copies.
                    ot_full = out_tiles.pop(pi)
                    nc.scalar.dma_start(
                        out_d.ap()[:, pi * OCHUNK:(pi + 1) * OCHUNK,
                                   :].rearrange("p k s -> p (k s)"),
                        ot_full[:].rearrange("p k s -> p (k s)"))

            loop_cm = (tc.For_i(0, reps, 1) if reps > 1
                       else contextlib.nullcontext())
            with loop_cm:
              for _inner in range(inner):
                chunk_tiles.clear()
                out_tiles.clear()
                for it in range(nh + 1):
                    if it < nh:
                        stage_tmp(it)
                        stage_mm(it)
                        stage_u(it)
                    if it >= 1:
                        stage_b(it - 1)

    nc.compile()
    return nc


def _prep_weights(qw1, qw2, kw1, kw2, qdd, kdd, w, tc_size=TC, ncores=NCORES):
    """Host-side weight folding. Returns per-core wbd + shared tiles."""
    nt = tc_size // TB
    wi = w[0].astype(np.float64) + np.eye(M)
    qw1f, qw2f = qw1[0, :, 0].astype(np.float64), qw2[0, :, 0].astype(np.float64)
    # W_t[m,n] = wi + sum_i qw1[t,i,m] qw2[t,i,n] + diag(qdd[t])
    Wt = wi[None] + np.einsum("tim,tin->tmn", qw1f, qw2f)
    Wt[:, np.arange(M), np.arange(M)] += qdd[0, :, 0].astype(np.float64)
    Wt = Wt.astype(np.float32)

    wbds = []
    for c in range(ncores):
        Wc = Wt[c * tc_size:(c + 1) * tc_size].reshape(nt, TB, M, M)
        wbd = np.zeros((nt, 128, 128), dtype=bf)
        for tb in range(TB):
            wbd[:, tb * M:(tb + 1) * M, tb * M:(tb + 1) * M] = Wc[:, tb].astype(bf)
        # pack partition-major: [128, nt, 128] (8 KB contiguous rows)
        wbds.append(np.ascontiguousarray(wbd.transpose(1, 0, 2)))

    rep = np.zeros((128, 128), dtype=bf)
    for tb in range(TB):
        rep[tb * M:(tb + 1) * M, tb * M:(tb + 1) * M] = 1.0
    idn = np.eye(128, dtype=np.float32).astype(bf)

    kw1f = kw1[0, :, 0]  # [S, I, M]
    kw2f = kw2[0, :, 0]
    kddf = kdd[0, :, 0]  # [S, M]
    ewin = np.empty((128, 3, S), dtype=bf)
    ewin[:, 0] = np.tile(kw1f[:, 0, :].T, (TB, 1)).astype(bf)
    ewin[:, 1] = np.tile(kw1f[:, 1, :].T, (TB, 1)).astype(bf)
    ewin[:, 2] = np.tile(kddf.T, (TB, 1)).astype(bf)
    ewout = np.empty((128, 2, S), dtype=bf)
    ewout[:, 0] = np.tile(kw2f[:, 0, :].T, (TB, 1)).astype(bf)
    ewout[:, 1] = np.tile(kw2f[:, 1, :].T, (TB, 1)).astype(bf)
    return wbds, rep, idn, ewin, ewout


def _make_in_maps(inputs, qw1, qw2, kw1, kw2, qdd, kdd, w,
                  tc_size=TC, ncores=NCORES):
    wbds, rep, idn, ewin, ewout = _prep_weights(
        qw1, qw2, kw1, kw2, qdd, kdd, w, tc_size, ncores
    )
    nt = tc_size // TB
    x = np.asarray(inputs)[0]  # [N, T, S] f32
    in_maps = []
    for c in range(ncores):
        xc = x[:, c * tc_size:(c + 1) * tc_size, :].astype(bf)  # [16, tc, S]
        # partition-major pack: xp[tb*16+m, ti, s] = xc[m, ti*TB+tb, s]
        xc = np.ascontiguousarray(
            xc.reshape(M, nt, TB, S).transpose(2, 0, 1, 3).reshape(128, nt, S)
        )
        in_maps.append({
            "x": xc, "wbd": wbds[c], "rep": rep, "idn": idn,
            "ewin": ewin, "ewout": ewout,
        })
    return in_maps


def kernel(inputs, qw1, qw2, kw1, kw2, qdd, kdd, w, trace=False):
    from concourse import bass_utils

    inputs = np.asarray(inputs, dtype=np.float32)
    qw1, qw2 = np.asarray(qw1, np.float32), np.asarray(qw2, np.float32)
    kw1, kw2 = np.asarray(kw1, np.float32), np.asarray(kw2, np.float32)
    qdd, kdd = np.asarray(qdd, np.float32), np.asarray(kdd, np.float32)
    w = np.asarray(w, np.float32)

    if "nc" not in _cache:
        _cache["nc"] = _build()
    nc = _cache["nc"]

    in_maps = _make_in_maps(inputs, qw1, qw2, kw1, kw2, qdd, kdd, w)
    res = bass_utils.run_bass_kernel_spmd(
        nc, in_maps, core_ids=list(range(NCORES)), trace=trace
    )
    nt = TC // TB
    outs = []
    for r in res.results:
        o = np.asarray(r["out"])  # [128, nt, S] bf16, partition-major
        # unpack: out[n, ti*TB+tb, s] = o[tb*16+n, ti, s]
        o = o.reshape(TB, M, nt, S).transpose(1, 2, 0, 3).reshape(M, TC, S)
        outs.append(o.astype(np.float32))
    out = np.concatenate(outs, axis=1)  # [N,T,S]
    _cache["last_results"] = res
    return out.reshape(B, N, T, S).astype(np.float32)
